# revision 45
# baseline (speedup 1.0000x reference)
"""Longformer self-attention on 8 Trainium2 NeuronCores (Bass/Tile).

nn_LongformerSelfAttention_65687229825616: B=2, T=2048, D=1024, H=16, hd=64,
WINDOW=128, DILATION=1, N_GLOBAL=1.

Sharding: token-parallel. Flattened query rows (B*T = 4096) are split into 8
contiguous slices of 512; each core computes all 16 heads for its slice and
the full output projection for its rows, so outputs concatenate with no
inter-core reduction. Each core's K/V cover its query range plus a 128-token
causal halo (left zero-padded at batch starts; padding is masked out).

The banded mask (query t attends keys [t-128, t] plus global key 0; query 0
attends everything) makes attention sparse: per 128-query block only a
256-key window + the global column. Scores are computed transposed (S^T,
[key, query] layout) so the PV matmul needs no on-chip transposes; softmax
column sums come from ones-selector matmuls accumulated into one [16, 512]
PSUM tile; reciprocals are broadcast back over partitions with an fp32
selector matmul. The global row 0 is handled with flash-style partials
(sumexp + unnormalized PV over each core's key slice) combined on the host.
All matmuls run in bf16 with fp32 PSUM accumulation.

Wall-clock strategy: the axon tunnel moves ~40 MB/s, so the four weight
matrices (8 MB bf16) are shipped once as a sharded stack and replicated
on-device with an XLA all-gather instead of 8 host copies; x ships as bf16;
input-independent constants (masks, selectors) are staged on device at
import; program build + walrus compile + jit warmup all happen at import.
Any device-path failure falls back to a numpy implementation.
"""
import sys

sys.path.insert(0, "/opt/trn_rl_repo")

import numpy as np
import ml_dtypes

BF16 = ml_dtypes.bfloat16

B, T, D, H, HD, W = 2, 2048, 1024, 16, 64, 128
NC = 8
NQ = 512
NKV = 640
NDT = 8
NLB = 4
SCALE = 1.0 / np.sqrt(HD)

_STATE = None
_INIT_ERR = None


# ---------------------------------------------------------------------------
# device program
# ---------------------------------------------------------------------------
def _build_program(upto="all"):
    import concourse.tile as tile
    from concourse import bacc, mybir

    STAGES = ("load", "stageA", "k0band", "row0", "pass1", "pass2", "stageC")
    lim = len(STAGES) if upto == "all" else STAGES.index(upto) + 1

    def on(stage):
        return STAGES.index(stage) < lim

    f32 = mybir.dt.float32
    b16 = mybir.dt.bfloat16
    Ident = mybir.ActivationFunctionType.Identity
    Exp = mybir.ActivationFunctionType.Exp

    nc = bacc.Bacc("TRN2", target_bir_lowering=False, debug=False,
                   enable_asserts=False, num_devices=NC)

    # per-core inputs
    xT_d = nc.dram_tensor("xT", [D, NKV], b16, kind="ExternalInput").ap()
    q0bd_d = nc.dram_tensor("q0bd", [NDT, 128, H], b16, kind="ExternalInput").ap()
    k0bd_d = nc.dram_tensor("k0bd", [NDT, 128, H], b16, kind="ExternalInput").ap()
    v0m_d = nc.dram_tensor("v0m", [H // 2, H, 128], b16, kind="ExternalInput").ap()
    mask_d = nc.dram_tensor("maskT", [4 * NLB, 128, 128], b16,
                            kind="ExternalInput").ap()
    k0m_d = nc.dram_tensor("k0mask", [H, NQ], b16, kind="ExternalInput").ap()
    # replicated inputs
    wall_d = nc.dram_tensor("wall", [4 * D, D], b16, kind="ExternalInput").ap()
    bqs_d = nc.dram_tensor("bqs", [128, NDT], f32, kind="ExternalInput").ap()
    bks_d = nc.dram_tensor("bks", [128, NDT], f32, kind="ExternalInput").ap()
    bv_d = nc.dram_tensor("bv", [1, D], b16, kind="ExternalInput").ap()
    bo_d = nc.dram_tensor("bo", [1, D], b16, kind="ExternalInput").ap()
    selr_d = nc.dram_tensor("selr", [H // 2, H, 128], f32,
                            kind="ExternalInput").ap()
    ocol_d = nc.dram_tensor("ocol", [128, H, H], b16, kind="ExternalInput").ap()

    out_d = nc.dram_tensor("out", [NQ, D], b16, kind="ExternalOutput").ap()
    r0pv_d = nc.dram_tensor("row0pv", [H, D], f32, kind="ExternalOutput").ap()
    r0s_d = nc.dram_tensor("row0sum", [1, H], f32, kind="ExternalOutput").ap()

    wq_dram = lambda di: wall_d[0 * D + di * 128:0 * D + (di + 1) * 128, :]
    wk_dram = lambda di: wall_d[1 * D + di * 128:1 * D + (di + 1) * 128, :]
    wv_dram = lambda di: wall_d[2 * D + di * 128:2 * D + (di + 1) * 128, :]
    wo_dram = lambda di: wall_d[3 * D + di * 128:3 * D + (di + 1) * 128, :]

    with tile.TileContext(nc) as tc:
        with (
            tc.tile_pool(name="const", bufs=1) as const,
            tc.tile_pool(name="data", bufs=1) as data,
            tc.tile_pool(name="cyc", bufs=3) as cyc,
            tc.tile_pool(name="psA", bufs=2, space="PSUM") as psA,
            tc.tile_pool(name="psS", bufs=3, space="PSUM") as psS,
            tc.tile_pool(name="psOT", bufs=2, space="PSUM") as psOT,
            tc.tile_pool(name="psSum", bufs=1, space="PSUM") as psSum,
        ):
            # DMA order matters: x + wq feed the first matmuls; each later
            # weight streams while the previous projection computes; wo is
            # not needed until stage C and loads last.
            xTb = []
            for di in range(NDT):
                xb = data.tile([128, NKV], b16, tag=f"xb{di}", name=f"xb{di}")
                nc.sync.dma_start(xb[:], xT_d[di * 128:(di + 1) * 128, :])
                xTb.append(xb)
            wq_sb, wk_sb, wv_sb, wo_sb = [], [], [], []
            for di in range(NDT):
                t = const.tile([128, D], b16, tag=f"wq{di}", name=f"wq{di}")
                nc.sync.dma_start(t[:], wq_dram(di))
                wq_sb.append(t)

            bqs_sb = const.tile([128, NDT], f32, tag="bqs")
            nc.sync.dma_start(bqs_sb[:], bqs_d[:])
            bks_sb = const.tile([128, NDT], f32, tag="bks")
            nc.sync.dma_start(bks_sb[:], bks_d[:])
            bv_sb = const.tile([1, D], b16, tag="bv")
            nc.sync.dma_start(bv_sb[:], bv_d[:])
            bo_sb = const.tile([1, D], b16, tag="bo")
            nc.sync.dma_start(bo_sb[:], bo_d[:])
            ones1_sb = const.tile([1, 128], b16, tag="ones1")
            nc.vector.memset(ones1_sb[:], 1.0)
            onesc_sb = const.tile([128, 1], b16, tag="onesc")
            nc.vector.memset(onesc_sb[:], 1.0)
            zrow_sb = const.tile([1, 512], b16, tag="zrow")
            nc.vector.memset(zrow_sb[:], 0.0)

            # ---- stage A: Q^T while wk streams, K^T while wv streams ----
            QT, KT = [], []
            for g in range(NDT if on("stageA") else 0):
                gsl = slice(g * 128, (g + 1) * 128)
                qt = data.tile([128, NQ], b16, tag=f"qt{g}", name=f"qt{g}")
                ps = psA.tile([128, NQ], f32, tag="psA")
                for di in range(NDT):
                    nc.tensor.matmul(ps[:], wq_sb[di][:, gsl], xTb[di][:, 128:NKV],
                                     start=di == 0, stop=di == NDT - 1)
                nc.vector.tensor_scalar_add(qt[:], ps[:], bqs_sb[:, g:g + 1])
                QT.append(qt)

            for di in range(NDT):
                t = const.tile([128, D], b16, tag=f"wk{di}", name=f"wk{di}")
                nc.sync.dma_start(t[:], wk_dram(di))
                wk_sb.append(t)
            for g in range(NDT if on("stageA") else 0):
                gsl = slice(g * 128, (g + 1) * 128)
                kt = data.tile([128, NKV], b16, tag=f"kt{g}", name=f"kt{g}")
                ps = psA.tile([128, NQ], f32, tag="psA")
                for di in range(NDT):
                    nc.tensor.matmul(ps[:], wk_sb[di][:, gsl], xTb[di][:, 0:NQ],
                                     start=di == 0, stop=di == NDT - 1)
                nc.scalar.activation(kt[:, 0:NQ], ps[:], func=Ident,
                                     bias=bks_sb[:, g:g + 1])
                ps2 = psA.tile([128, 128], f32, tag="psA")
                for di in range(NDT):
                    nc.tensor.matmul(ps2[:], wk_sb[di][:, gsl], xTb[di][:, NQ:NKV],
                                     start=di == 0, stop=di == NDT - 1)
                nc.scalar.activation(kt[:, NQ:NKV], ps2[:], func=Ident,
                                     bias=bks_sb[:, g:g + 1])
                KT.append(kt)

            for di in range(NDT):
                t = const.tile([128, D], b16, tag=f"wv{di}", name=f"wv{di}")
                nc.sync.dma_start(t[:], wv_dram(di))
                wv_sb.append(t)
            q0bd_sb = const.tile([128, NDT, H], b16, tag="q0bd")
            nc.sync.dma_start(q0bd_sb[:], q0bd_d.rearrange("g p c -> p g c"))
            k0bd_sb = const.tile([128, NDT, H], b16, tag="k0bd")
            nc.sync.dma_start(k0bd_sb[:], k0bd_d.rearrange("g p c -> p g c"))
            v0m_sb = const.tile([H, H // 2, 128], b16, tag="v0m")
            nc.sync.dma_start(v0m_sb[:], v0m_d.rearrange("g p q -> p g q"))
            selr_sb = const.tile([H, H // 2, 128], f32, tag="selr")
            nc.sync.dma_start(selr_sb[:], selr_d.rearrange("g p q -> p g q"))
            ocol_sb = const.tile([128, H, H], b16, tag="ocol")
            nc.sync.dma_start(ocol_sb[:], ocol_d[:])
            mask_sb = const.tile([128, 4 * NLB, 128], b16, tag="mask")
            nc.sync.dma_start(mask_sb[:], mask_d.rearrange("m p q -> p m q"))
            k0m_sb = const.tile([H, NQ], b16, tag="k0m")
            nc.sync.dma_start(k0m_sb[:], k0m_d[:])

            V = []
            for v in range(5 if on("stageA") else 0):
                vt = data.tile([128, D], b16, tag=f"v{v}", name=f"v{v}")
                for nh in range(2):
                    nsl = slice(nh * 512, (nh + 1) * 512)
                    ps = psA.tile([128, 512], f32, tag="psA")
                    for di in range(NDT):
                        nc.tensor.matmul(ps[:], xTb[di][:, v * 128:(v + 1) * 128],
                                         wv_sb[di][:, nsl],
                                         start=di == 0, stop=False)
                    nc.tensor.matmul(ps[:], ones1_sb[0:1, :], bv_sb[0:1, nsl],
                                     start=False, stop=True)
                    nc.scalar.copy(vt[:, nsl], ps[:])
                V.append(vt)

            # wo last: stage C runs near the end of the kernel
            for di in range(NDT):
                t = const.tile([128, D], b16, tag=f"wo{di}", name=f"wo{di}")
                nc.sync.dma_start(t[:], wo_dram(di))
                wo_sb.append(t)

            # ---- global-column scores p0band[h, t] = exp(q_t.k_0)*k0mask ----
            if on("k0band"):
                s0b = psA.tile([H, NQ], f32, tag="psA")
                for g in range(NDT):
                    nc.tensor.matmul(s0b[:], k0bd_sb[:, g, :], QT[g][:],
                                     start=g == 0, stop=g == NDT - 1)
                p0e = cyc.tile([H, NQ], b16, tag="p0e")
                nc.scalar.activation(p0e[:], s0b[:], func=Exp)
                p0band = data.tile([H, NQ], b16, tag="p0band")
                nc.vector.tensor_mul(p0band[:], p0e[:], k0m_sb[:])

            # ---- global row 0: flash partials over this core's key slice ----
            # scores computed directly transposed: s0T[t, h] via block-diag
            # q0bd as rhs — no PE transposes needed.
            if on("row0"):
                p0T = cyc.tile([128, 4, H], b16, tag="p0T")
                for tch in range(4):
                    s0t = psS.tile([128, H], f32, tag="psS")
                    tsl = slice(128 + tch * 128, 128 + (tch + 1) * 128)
                    for g in range(NDT):
                        nc.tensor.matmul(s0t[:], KT[g][:, tsl],
                                         q0bd_sb[:, g, :],
                                         start=g == 0, stop=g == NDT - 1)
                    nc.scalar.activation(p0T[:, tch, :], s0t[:], func=Exp)
                rs = psS.tile([1, H], f32, tag="psS")
                for tch in range(4):
                    nc.tensor.matmul(rs[:], onesc_sb[:], p0T[:, tch, :],
                                     start=tch == 0, stop=tch == 3)
                r0sum = cyc.tile([1, H], f32, tag="r0sum")
                nc.vector.tensor_copy(r0sum[:], rs[:])
                nc.sync.dma_start(r0s_d[:], r0sum[:])
                for nh in range(2):
                    nsl = slice(nh * 512, (nh + 1) * 512)
                    ps = psA.tile([H, 512], f32, tag="psA")
                    for tch in range(4):
                        nc.tensor.matmul(ps[:], p0T[:, tch, :], V[1 + tch][:, nsl],
                                         start=tch == 0, stop=tch == 3)
                    pv = cyc.tile([H, 512], f32, tag="pvf")
                    nc.vector.tensor_copy(pv[:], ps[:])
                    nc.sync.dma_start(r0pv_d[:, nsl], pv[:])

            # ---- band pass 1: S^T -> exp -> mask -> P; column sums ----
            # head-pair layout: one [128, 512] PSUM holds S^T for heads
            # (2g, 2g+1): cols [(sub*2+kh)*128, ...). sub=1 matmuls use
            # partition base 64 -> distinct PE row groups run concurrently.
            sums = psSum.tile([H, NQ], f32, tag="sums")
            mask_flat = mask_sb[:].rearrange("p a b -> p (a b)")
            if on("pass1"):
                # full-region zeroing matmul: forces WAW ordering before the
                # column-sliced accumulating matmuls below (the scheduler may
                # otherwise reorder disjoint-region writes).
                nc.tensor.matmul(sums[:], zrow_sb[0:1, 0:H], zrow_sb[0:1, :],
                                 start=True, stop=False, skip_group_check=True)

            P = []
            for g in range(NDT if on("pass1") else 0):
                ph = data.tile([128, NLB, 512], b16, tag=f"p{g}", name=f"p{g}")
                for lb in range(NLB):
                    st = psS.tile([128, 512], f32, tag="psS")
                    for sub in range(2):
                        po = sub * 64
                        for kh in range(2):
                            # disjoint single-write column regions: flags are
                            # data-safe under any order; skip the group check
                            nc.tensor.matmul(
                                st[:, (sub * 2 + kh) * 128:(sub * 2 + kh + 1) * 128],
                                KT[g][po:po + 64,
                                      (lb + kh) * 128:(lb + kh + 1) * 128],
                                QT[g][po:po + 64, lb * 128:(lb + 1) * 128],
                                start=(sub == 0 and kh == 0),
                                stop=(sub == 1 and kh == 1),
                                skip_group_check=True)
                    pe = cyc.tile([128, 512], b16, tag="pe")
                    nc.scalar.activation(pe[:], st[:], func=Exp)
                    nc.vector.tensor_mul(ph[:, lb, :], pe[:],
                                         mask_flat[:, lb * 512:(lb + 1) * 512])
                    for sub in range(2):
                        for kh in range(2):
                            nc.tensor.matmul(
                                sums[:, lb * 128:(lb + 1) * 128],
                                ocol_sb[:, 2 * g + sub, :],
                                ph[:, lb, (sub * 2 + kh) * 128:
                                   (sub * 2 + kh + 1) * 128],
                                start=False, stop=False, skip_group_check=True)
                P.append(ph)

            if on("pass1"):
                nc.vector.tensor_add(sums[:], sums[:], p0band[:])
                recip = data.tile([H, NQ], f32, tag="recip")
                nc.vector.reciprocal(recip[:], sums[:])

            # ---- band pass 2: PV (head-pair packed), k0 term, normalize ----
            attnT = [data.tile([128, NQ], b16, tag=f"at{g}", name=f"at{g}")
                     for g in range(NDT)]
            for g in range(NDT if on("pass2") else 0):
                ot = psOT.tile([128, NQ], f32, tag="ot")
                # v0m term first: it writes the FULL tile with start=True, so
                # every later PV matmul has a WAW dep on it (ordering) and
                # accumulates onto cleared elements.
                nc.tensor.matmul(ot[:], v0m_sb[:, g, :], p0band[:],
                                 start=True, stop=False, skip_group_check=True)
                for sub in range(2):
                    h, po = 2 * g + sub, sub * 64
                    for lb in range(NLB):
                        for kh in range(2):
                            nc.tensor.matmul(
                                ot[po:po + 64, lb * 128:(lb + 1) * 128],
                                V[lb + kh][:, h * 64:(h + 1) * 64],
                                P[g][:, lb, (sub * 2 + kh) * 128:
                                     (sub * 2 + kh + 1) * 128],
                                start=False, stop=False, skip_group_check=True)
                bc = psOT.tile([128, NQ], f32, tag="ot")
                nc.tensor.matmul(bc[:], selr_sb[:, g, :], recip[:],
                                 start=True, stop=True)
                osb = cyc.tile([128, NQ], b16, tag="osb")
                nc.scalar.copy(osb[:], ot[:])
                nc.vector.tensor_mul(attnT[g][:], osb[:], bc[:])

            # ---- stage C: output projection ----
            for tt in range(NLB if on("stageC") else 0):
                tsl = slice(tt * 128, (tt + 1) * 128)
                for nh in range(2):
                    nsl = slice(nh * 512, (nh + 1) * 512)
                    ps = psA.tile([128, 512], f32, tag="psA")
                    for di in range(NDT):
                        nc.tensor.matmul(ps[:], attnT[di][:, tsl],
                                         wo_sb[di][:, nsl],
                                         start=di == 0, stop=False)
                    nc.tensor.matmul(ps[:], ones1_sb[0:1, :], bo_sb[0:1, nsl],
                                     start=False, stop=True)
                    ob = cyc.tile([128, 512], b16, tag="ob")
                    if (tt + nh) % 2:
                        nc.scalar.copy(ob[:], ps[:])
                    else:
                        nc.vector.tensor_copy(ob[:], ps[:])
                    nc.sync.dma_start(out_d[tsl, nsl], ob[:])

    nc.compile()
    return nc


# names/order must match the jit argument order below
_SHARDED = ("xT", "q0bd", "k0bd", "v0m", "maskT", "k0mask")
_REPL = ("wall", "bqs", "bks", "bv", "bo", "selr", "ocol")
_OUTS = ("out", "row0pv", "row0sum")


def _init():
    """Build + compile the program, create jits, stage constants, warm up."""
    import jax
    import jax.numpy as jnp
    from jax.sharding import Mesh, PartitionSpec, NamedSharding
    from jax.experimental.shard_map import shard_map
    from concourse import bass2jax, mybir

    nc = _build_program()
    bass2jax.install_neuronx_cc_hook()

    devices = jax.devices()[:NC]
    mesh = Mesh(np.asarray(devices), ("core",))
    P_core = PartitionSpec("core")
    P_repl = PartitionSpec()
    sh_core = NamedSharding(mesh, P_core)
    sh_repl = NamedSharding(mesh, P_repl)

    # collect BIR I/O metadata
    in_shapes = {}
    out_avals = []
    out_shapes = {}
    for alloc in nc.m.functions[0].allocations:
        if not isinstance(alloc, mybir.MemoryLocationSet):
            continue
        name = alloc.memorylocations[0].name
        shape = tuple(alloc.tensor_shape)
        dtype = mybir.dt.np(alloc.dtype)
        if alloc.kind == "ExternalInput":
            in_shapes[name] = (shape, dtype)
        elif alloc.kind == "ExternalOutput":
            out_shapes[name] = (shape, dtype)
    for name in _OUTS:
        shape, dtype = out_shapes[name]
        out_avals.append(jax.core.ShapedArray(shape, dtype))

    in_names = list(_SHARDED) + list(_REPL) + list(_OUTS)
    if nc.partition_id_tensor is not None:
        in_names.append(nc.partition_id_tensor.name)

    def _body(*args):
        ops = list(args)
        if nc.partition_id_tensor is not None:
            ops.append(bass2jax.partition_id_tensor())
        outs = bass2jax._bass_exec_p.bind(
            *ops, out_avals=tuple(out_avals), in_names=tuple(in_names),
            out_names=tuple(_OUTS), lowering_input_output_aliases=(),
            sim_require_finite=True, sim_require_nnan=True, nc=nc)
        return tuple(outs)

    n_shard = len(_SHARDED)
    n_repl = len(_REPL)
    n_out = len(_OUTS)
    in_specs = (P_core,) * n_shard + (P_repl,) * n_repl + (P_core,) * n_out
    out_specs = (P_core,) * n_out
    main_fn = jax.jit(
        shard_map(_body, mesh=mesh, in_specs=in_specs, out_specs=out_specs,
                  check_rep=False),
        donate_argnums=tuple(range(n_shard + n_repl, n_shard + n_repl + n_out)),
        keep_unused=True)

    # one dispatch: weight all-gather ([8, 512, 1024] shards -> replicated
    # [4096, 1024]) + on-device output zero buffers (donated into main_fn)
    def _gather_zeros(v):
        g = shard_map(lambda w: jax.lax.all_gather(w, "core", axis=0, tiled=True),
                      mesh=mesh, in_specs=(PartitionSpec("core", None, None),),
                      out_specs=P_repl, check_rep=False)(v)
        zs = tuple(
            jnp.zeros((NC * out_shapes[n][0][0], *out_shapes[n][0][1:]),
                      out_shapes[n][1]) for n in _OUTS)
        return (g,) + zs
    gather_fn = jax.jit(_gather_zeros,
                        out_shardings=(sh_repl,) + (sh_core,) * n_out)

    # input-independent per-core constants, staged once
    def _dup_sub(m):  # [2*NLB,128,128] (lb,kh) -> [4*NLB,128,128] (lb,sub,kh)
        m = m.reshape(NLB, 2, 128, 128)
        return np.repeat(m[:, None], 2, axis=1).reshape(4 * NLB, 128, 128)

    mask_first = _dup_sub(_build_mask(True)).astype(BF16)
    mask_rest = _dup_sub(_build_mask(False)).astype(BF16)
    k0m_first = np.broadcast_to(
        (np.arange(NQ) >= 256).astype(np.float32), (H, NQ)).astype(BF16)
    k0m_rest = np.ones((H, NQ), BF16)
    maskT_np = np.concatenate(
        [mask_first if c % 4 == 0 else mask_rest for c in range(NC)], axis=0)
    k0m_np = np.concatenate(
        [k0m_first if c % 4 == 0 else k0m_rest for c in range(NC)], axis=0)
    selr = np.zeros((H // 2, H, 128), np.float32)
    for g in range(H // 2):
        selr[g, 2 * g, 0:64] = 1.0
        selr[g, 2 * g + 1, 64:128] = 1.0
    ocol = np.zeros((128, H, H), np.float32)
    for h in range(H):
        ocol[:, h, h] = 1.0

    maskT_dev = jax.device_put(maskT_np, sh_core)
    k0m_dev = jax.device_put(k0m_np, sh_core)
    selr_dev = jax.device_put(selr, sh_repl)
    ocol_dev = jax.device_put(ocol.astype(BF16), sh_repl)

    state = dict(nc=nc, jax=jax, mesh=mesh, sh_core=sh_core, sh_repl=sh_repl,
                 main_fn=main_fn, gather_fn=gather_fn,
                 maskT=maskT_dev, k0mask=k0m_dev, selr=selr_dev, ocol=ocol_dev,
                 out_shapes=out_shapes)

    # warmup: compile everything once with dummy data
    dummy = {k: np.zeros_like(np.empty(s, d))
             for k, (s, d) in in_shapes.items() if k != "partition_id"}
    ins = {
        "x": np.zeros((B, T, D), np.float32),
        "Wq": np.zeros((D, D), np.float32), "bq": np.zeros(D, np.float32),
        "Wk": np.zeros((D, D), np.float32), "bk": np.zeros(D, np.float32),
        "Wv": np.zeros((D, D), np.float32), "bv": np.zeros(D, np.float32),
        "Wo": np.eye(D, dtype=np.float32), "bo": np.zeros(D, np.float32),
    }
    _run_device(state, **ins)
    return state


def _build_mask(first_core):
    m = np.zeros((NLB, 2, 128, 128), np.float32)
    j = np.arange(256)[:, None]
    r = np.arange(128)[None, :]
    band = (j >= r) & (j <= r + 128)
    for lb in range(NLB):
        if first_core:
            kabs = (lb - 1) * 128 + j
            valid = (band & (kabs >= 0)) | (kabs == 0)
        else:
            valid = band
        m[lb, 0] = valid[:128]
        m[lb, 1] = valid[128:]
    return m.reshape(2 * NLB, 128, 128)


def _prep_host(x, Wq, bq, Wk, bk, Wv, bv, Wo, bo):
    """Host-side packing shared by the device path and CoreSim debugging.

    Returns (wall [4D, D] bf16, sharded: dict name->concat array, repl: dict).
    """
    f32 = np.float32
    x = np.asarray(x, f32)
    Wq, Wk, Wv, Wo = (np.asarray(a, f32) for a in (Wq, Wk, Wv, Wo))
    bq, bk, bv, bo = (np.asarray(a, f32) for a in (bq, bk, bv, bo))

    wall = np.empty((4 * D, D), BF16)
    wall[0:D] = Wq.T * SCALE
    wall[D:2 * D] = Wk.T
    wall[2 * D:3 * D] = Wv.T
    wall[3 * D:] = Wo.T

    repl = {
        "bqs": np.ascontiguousarray((bq * SCALE).reshape(NDT, 128).T),
        "bks": np.ascontiguousarray(bk.reshape(NDT, 128).T),
        "bv": bv.reshape(1, D).astype(BF16),
        "bo": bo.reshape(1, D).astype(BF16),
    }

    xb16 = [np.ascontiguousarray(x[b].astype(BF16).T) for b in range(B)]  # [D,T]
    xT_np = np.empty((NC * D, NKV), BF16)
    for c in range(NC):
        b, jj = divmod(c, NC // B)
        s = jj * NQ
        dst = xT_np[c * D:(c + 1) * D]
        if jj == 0:
            dst[:, :128] = 0
            dst[:, 128:] = xb16[b][:, :NQ]
        else:
            dst[:] = xb16[b][:, s - 128:s + NQ]

    q0bd = np.zeros((B, NDT, 128, H), f32)
    k0bd = np.zeros((B, NDT, 128, H), f32)
    v0m = np.zeros((B, H // 2, H, 128), f32)
    for b in range(B):
        q0 = (x[b, 0] @ Wq.T + bq) * SCALE
        k0 = x[b, 0] @ Wk.T + bk
        v0 = x[b, 0] @ Wv.T + bv
        for h in range(H):
            g, po = h // 2, (h % 2) * 64
            q0bd[b, g, po:po + 64, h] = q0[h * 64:(h + 1) * 64]
            k0bd[b, g, po:po + 64, h] = k0[h * 64:(h + 1) * 64]
            v0m[b, g, h, po:po + 64] = v0[h * 64:(h + 1) * 64]
    bidx = [c // (NC // B) for c in range(NC)]
    sharded = {
        "xT": xT_np,
        "q0bd": np.concatenate([q0bd[b] for b in bidx], axis=0).astype(BF16),
        "k0bd": np.concatenate([k0bd[b] for b in bidx], axis=0).astype(BF16),
        "v0m": np.concatenate([v0m[b] for b in bidx], axis=0).astype(BF16),
    }
    return wall, sharded, repl


def _run_device(state, x, Wq, bq, Wk, bk, Wv, bv, Wo, bo):
    jax = state["jax"]
    f32 = np.float32
    Wo_f = np.asarray(Wo, f32)
    bo_f = np.asarray(bo, f32)

    wall, sharded, repl = _prep_host(x, Wq, bq, Wk, bk, Wv, bv, Wo, bo)
    wall_sh = jax.device_put(wall.reshape(NC, 4 * D // NC, D), state["sh_core"])
    wall_dev, *zeros = state["gather_fn"](wall_sh)   # async

    small_repl = [
        jax.device_put(repl["bqs"], state["sh_repl"]),
        jax.device_put(repl["bks"], state["sh_repl"]),
        jax.device_put(repl["bv"], state["sh_repl"]),
        jax.device_put(repl["bo"], state["sh_repl"]),
        state["selr"], state["ocol"],
    ]
    xT_dev = jax.device_put(sharded["xT"], state["sh_core"])
    q0bd_dev = jax.device_put(sharded["q0bd"], state["sh_core"])
    k0bd_dev = jax.device_put(sharded["k0bd"], state["sh_core"])
    v0m_dev = jax.device_put(sharded["v0m"], state["sh_core"])

    outs = state["main_fn"](
        xT_dev, q0bd_dev, k0bd_dev, v0m_dev, state["maskT"], state["k0mask"],
        wall_dev, *small_repl, *zeros)

    out, r0pv, r0sum = (np.asarray(o) for o in outs)
    out = out.astype(np.float32).reshape(B, T, D)
    r0pv = r0pv.reshape(NC, H, D)
    r0sum = r0sum.reshape(NC, H)

    for b in range(B):
        cores = range(b * (NC // B), (b + 1) * (NC // B))
        ssum = np.zeros(H, np.float64)
        pv = np.zeros((H, HD), np.float64)
        for c in cores:
            ssum += r0sum[c].astype(np.float64)
            for h in range(H):
                pv[h] += r0pv[c, h, h * 64:(h + 1) * 64]
        attn0 = (pv / ssum[:, None]).reshape(D).astype(f32)
        out[b, 0, :] = attn0 @ Wo_f.T + bo_f

    return out.astype(f32)


def _numpy_kernel(x, Wq, bq, Wk, bk, Wv, bv, Wo, bo):
    """Host fallback (same math as the reference, banded)."""
    f32 = np.float32
    x = np.asarray(x, f32)
    Wq, Wk, Wv, Wo = (np.asarray(a, f32) for a in (Wq, Wk, Wv, Wo))
    bq, bk, bv, bo = (np.asarray(a, f32) for a in (bq, bk, bv, bo))
    NB = T // 128
    xf = x.reshape(B * T, D)

    def heads(t):
        return t.reshape(B, T, H, HD).transpose(0, 2, 1, 3)

    Q = heads((xf @ Wq.T + bq) * SCALE)
    K = heads(xf @ Wk.T + bk)
    Vm = heads(xf @ Wv.T + bv)
    pad = np.zeros((B, H, 128, HD), f32)
    Kp = np.concatenate([pad, K], axis=2)
    Vp = np.concatenate([pad, Vm], axis=2)
    Qb = Q.reshape(B, H, NB, 128, HD)
    widx = 128 * np.arange(NB)[:, None] + np.arange(256)[None, :]
    Kwin = Kp[:, :, widx]
    Vwin = Vp[:, :, widx]
    s = np.empty((B, H, NB, 128, 257), f32)
    np.einsum('bhnqd,bhnkd->bhnqk', Qb, Kwin, out=s[..., 1:257], optimize=True)
    s[..., 0] = np.einsum('bhnqd,bhd->bhnq', Qb, K[:, :, 0], optimize=True)
    r = np.arange(128)[:, None]
    j = np.arange(257)[None, :]
    NEG = f32(-1e9)
    band = (j >= r + 1) & (j <= r + 129)
    maskN = np.where((j == 0) | band, 0.0, NEG).astype(f32)
    mask1 = np.where((j == 1) | band, 0.0, NEG).astype(f32)
    mask0 = np.where((j >= 129) & (j <= 129 + r), 0.0, NEG).astype(f32)
    s += maskN
    s[:, :, 0] += mask0 - maskN
    s[:, :, 1] += mask1 - maskN
    np.clip(s, -60.0, None, out=s)
    ae = np.exp(s)
    ssum = ae.sum(axis=-1, keepdims=True)
    o = np.einsum('bhnqk,bhnkd->bhnqd', ae[..., 1:257], Vwin, optimize=True)
    o += ae[..., 0:1] * Vm[:, :, None, None, 0]
    o /= ssum
    attn = o.reshape(B, H, T, HD)
    s0 = np.einsum('bhd,bhtd->bht', Q[:, :, 0], K, optimize=True)
    a0 = np.exp(s0 - s0.max(axis=-1, keepdims=True))
    attn[:, :, 0] = np.einsum('bht,bhtd->bhd', a0, Vm, optimize=True) \
        / a0.sum(axis=-1, keepdims=True)
    out = attn.transpose(0, 2, 1, 3).reshape(B * T, D) @ Wo.T + bo
    return out.reshape(B, T, D).astype(f32)


def kernel(x, Wq, bq, Wk, bk, Wv, bv, Wo, bo):
    global _STATE, _INIT_ERR
    if _STATE is None and _INIT_ERR is None:
        try:
            _STATE = _init()
        except Exception as e:  # pragma: no cover - safety net
            _INIT_ERR = e
            import traceback
            traceback.print_exc()
    if _STATE is not None:
        try:
            return _run_device(_STATE, x, Wq, bq, Wk, bk, Wv, bv, Wo, bo)
        except Exception:  # pragma: no cover - safety net
            import traceback
            traceback.print_exc()
    return _numpy_kernel(x, Wq, bq, Wk, bk, Wv, bv, Wo, bo)


# build + compile + warm up at import so the first kernel() call is fast
try:
    _STATE = _init()
except Exception as e:  # pragma: no cover - safety net
    _INIT_ERR = e
